# revision 8
# baseline (speedup 1.0000x reference)
"""Trainium2 Bass kernel for nn_BiGCN (3-layer GCN: batchnorm -> 3x [adj @ (x W) + b] with
dropout between layers, final log_softmax).

Strategy (8 NeuronCores, SPMD):
  - Row-shard adj over N: core c owns rows [c*1250, (c+1)*1250).
  - Host pre-transposes: each core receives adjT_c = adj[rows_c, :].T  (shape [N, 1250]),
    zero-padded per 1250-column block to 1280 so the contraction dim tiles evenly by 128,
    cast to bf16.  x is sent transposed ([512, 1250] slice, fp32).
  - On device everything is computed in the "transposed" orientation z^T = y^T @ adjT so
    per-feature bias/normalization are per-partition ops, and the y = d @ W matmuls take d^T
    directly as lhsT (no transposes anywhere except the final log_softmax).
  - Between layers, each core computes its row-block of y = d @ W in fp32, quantizes to bf16
    for the big adj matmul, and the blocks are exchanged with an on-device AllGather.
  - Numerical structure: adj has near-constant row sums (~N/2), which amplify the column-mean
    of any quantization error of y by ~100x per layer (compounding).  We therefore keep the
    d/y path in fp32 and add a rank-1 compensation rowsum_i * colmean(y - yq)_f to each adj
    matmul (one K=1 matmul accumulated into the same PSUM bank).  Validated to ~2e-3 final
    relative error vs the fp32 reference (naive bf16 is ~0.2).
  - Batchnorm statistics use a tiny AllReduce of per-core partial sums.
"""

import numpy as np
import ml_dtypes

import concourse.bass as bass
import concourse.bacc as bacc
import concourse.mybir as mybir
from concourse import tile, masks
from concourse.bass_utils import run_bass_kernel_spmd

f32 = mybir.dt.float32
bf16 = mybir.dt.bfloat16
AF = mybir.ActivationFunctionType
ALU = mybir.AluOpType
AX = mybir.AxisListType

# ---- problem shape (hardcoded per spec) ----
N = 10000
IN_C, HID, MID, OUT_C = 512, 256, 128, 64
EPS = 1e-5
NCORES = 8
RPC = N // NCORES          # 1250 rows per core
KBLK = 1280                # padded contraction rows per core block (10 k-tiles)
KPAD = NCORES * KBLK       # 10240
KT = KPAD // 128           # 80 k-tiles
BLK = KBLK + 1             # gather block: 1280 y rows (1250 real + 30 zero) + 1 colsum row
NT = [(0, 512), (512, 512), (1024, 226)]      # free-dim tiles of the 1250 row block
MTS = [128] * 9 + [98]                        # row tiles of the 1250 rows (sum = 1250)

# ---- tunables ----
R_RES = 32                 # adj k-tiles kept resident in SBUF across all three layers
STREAM_BUFS = 8
YK_BUFS = 6

# real (non-padded) contraction rows covered by the resident tiles (for the partial rowsum
# estimate used by the rank-1 compensation; estimate noise is iid and harmless)
def _covered_real(r_res: int) -> int:
    cov = 0
    for k in range(r_res):
        t = k % 10
        lo, hi = t * 128, min(t * 128 + 128, RPC)
        cov += max(0, hi - lo)
    return cov


COVERED = _covered_real(R_RES)

_BUILT = None


def _build():
    nc = bacc.Bacc("TRN2", target_bir_lowering=False, debug=False, num_devices=NCORES)

    adjT = nc.dram_tensor("adjT", [KPAD, RPC], bf16, kind="ExternalInput")
    xT = nc.dram_tensor("xT", [IN_C, RPC], f32, kind="ExternalInput")
    m0T = nc.dram_tensor("m0T", [HID, RPC], f32, kind="ExternalInput")
    m1T = nc.dram_tensor("m1T", [MID, RPC], f32, kind="ExternalInput")
    W0d = nc.dram_tensor("W0", [IN_C, HID], f32, kind="ExternalInput")
    W1d = nc.dram_tensor("W1", [HID, MID], f32, kind="ExternalInput")
    W2d = nc.dram_tensor("W2", [MID, OUT_C], f32, kind="ExternalInput")
    b0d = nc.dram_tensor("b0", [HID, 1], f32, kind="ExternalInput")
    b1d = nc.dram_tensor("b1", [MID, 1], f32, kind="ExternalInput")
    b2d = nc.dram_tensor("b2", [OUT_C, 1], f32, kind="ExternalInput")
    outd = nc.dram_tensor("out", [RPC, OUT_C], f32, kind="ExternalOutput")

    with tile.TileContext(nc) as tc:
        with (
            tc.tile_pool(name="const", bufs=1) as const,
            tc.tile_pool(name="xp", bufs=1) as xp,
            tc.tile_pool(name="dp", bufs=1) as dp,
            tc.tile_pool(name="mp", bufs=1) as mp,
            tc.tile_pool(name="res", bufs=1) as res,
            tc.tile_pool(name="stream", bufs=1) as stream,
            tc.tile_pool(name="stage", bufs=1) as stage,
            tc.tile_pool(name="ps", space="PSUM", bufs=1) as ps,
            tc.tile_pool(name="dram", space="DRAM", bufs=1) as dram,
        ):
            rg = [list(range(NCORES))]

            # ---------- constants ----------
            ident = const.tile([64, 64], f32)
            masks.make_identity(nc, ident[:])
            ones_col = const.tile([128, 1], bf16)
            nc.gpsimd.memset(ones_col[:], 1.0)
            zrow = const.tile([32, HID], bf16)
            nc.gpsimd.memset(zrow[:], 0.0)

            W0s = const.tile([128, 4 * HID], f32)
            for kk in range(4):
                nc.sync.dma_start(W0s[:, kk * HID:(kk + 1) * HID],
                                  W0d[kk * 128:(kk + 1) * 128, :])
            W1s = const.tile([128, 2 * MID], f32)
            for kk in range(2):
                nc.sync.dma_start(W1s[:, kk * MID:(kk + 1) * MID],
                                  W1d[kk * 128:(kk + 1) * 128, :])
            W2s = const.tile([128, OUT_C], f32)
            nc.sync.dma_start(W2s[:], W2d[:])
            b0s = const.tile([128, 2], f32)
            nc.sync.dma_start(b0s[:, 0:1], b0d[0:128, :])
            nc.sync.dma_start(b0s[:, 1:2], b0d[128:256, :])
            b1s = const.tile([128, 1], f32)
            nc.sync.dma_start(b1s[:], b1d[:])
            b2s = const.tile([64, 1], f32)
            nc.sync.dma_start(b2s[:], b2d[:])

            # ---------- resident adj tiles + partial rowsum ----------
            res_tiles = []
            for k in range(R_RES):
                t = res.tile([128, RPC], bf16, name=f"ar{k}", tag=f"ar{k}")
                nc.sync.dma_start(t[:], adjT[k * 128:(k + 1) * 128, :])
                res_tiles.append(t)

            rs_ps = [ps.tile([1, n], f32, name=f"rsps{i}", tag="z", bufs=6)
                     for i, (o, n) in enumerate(NT)]
            for k in range(R_RES):
                for i, (o, n) in enumerate(NT):
                    nc.tensor.matmul(rs_ps[i][:], ones_col[:, :], res_tiles[k][:, o:o + n],
                                     start=(k == 0), stop=(k == R_RES - 1))
            # compensation = rowsum_est_i * colmean_f = (partial_i*N/COVERED) * (colsum_f/N)
            #              = partial_i * colsum_f / COVERED; the 1/COVERED lives here.
            rowsumT = const.tile([1, RPC], bf16)
            for i, (o, n) in enumerate(NT):
                nc.scalar.activation(rowsumT[:, o:o + n], rs_ps[i][:], AF.Copy,
                                     scale=1.0 / COVERED)

            # ---------- batchnorm statistics ----------
            xk = []
            for i in range(4):
                t = xp.tile([128, RPC], f32, name=f"xk{i}", tag=f"xk{i}")
                nc.sync.dma_start(t[:], xT[i * 128:(i + 1) * 128, :])
                xk.append(t)

            stats = const.tile([128, 8], f32)
            sq = dp.tile([128, RPC], f32, name="sq", tag="d1a")
            for i in range(4):
                nc.vector.reduce_sum(stats[:, i:i + 1], xk[i][:], axis=AX.X)
                nc.scalar.activation(sq[:], xk[i][:], AF.Square,
                                     accum_out=stats[:, 4 + i:5 + i])

            bn_in = dram.tile([128, 8], f32)
            bn_out = dram.tile([128, 8], f32, addr_space="Shared")
            nc.sync.dma_start(bn_in[:], stats[:])
            nc.gpsimd.collective_compute("AllReduce", ALU.add, replica_groups=rg,
                                         ins=[bn_in[:]], outs=[bn_out[:]])
            statsr = const.tile([128, 8], f32)
            nc.sync.dma_start(statsr[:], bn_out[:])

            meanv = const.tile([128, 4], f32)
            ex2 = const.tile([128, 4], f32)
            var = const.tile([128, 4], f32)
            sd = const.tile([128, 4], f32)
            inv = const.tile([128, 4], f32)
            shift = const.tile([128, 4], f32)
            nc.vector.tensor_scalar_mul(meanv[:], statsr[:, 0:4], 1.0 / N)
            nc.vector.tensor_scalar_mul(ex2[:], statsr[:, 4:8], 1.0 / N)
            nc.vector.tensor_mul(var[:], meanv[:], meanv[:])
            nc.vector.tensor_sub(var[:], ex2[:], var[:])
            nc.vector.tensor_scalar_add(var[:], var[:], EPS)
            nc.scalar.activation(sd[:], var[:], AF.Sqrt)
            nc.vector.reciprocal(inv[:], sd[:])
            nc.vector.tensor_mul(shift[:], meanv[:], inv[:])
            nc.vector.tensor_scalar_mul(shift[:], shift[:], -1.0)

            # normalize x in place (per-partition scale/bias)
            for i in range(4):
                nc.scalar.activation(xk[i][:], xk[i][:], AF.Identity,
                                     scale=inv[:, i:i + 1], bias=shift[:, i:i + 1])

            # ---------- gather buffers ----------
            yg_in = [dram.tile([BLK, HID], bf16, name="ygi0"),
                     dram.tile([BLK, MID], bf16, name="ygi1"),
                     dram.tile([BLK, OUT_C], bf16, name="ygi2")]
            yg_out = [dram.tile([NCORES * BLK, HID], bf16, addr_space="Shared", name="ygo0"),
                      dram.tile([NCORES * BLK, MID], bf16, addr_space="Shared", name="ygo1"),
                      dram.tile([NCORES * BLK, OUT_C], bf16, addr_space="Shared", name="ygo2")]

            def produce_y(idx, f_out, lhs_tiles, rhs_w, kparts):
                """Compute y = d @ W (fp32), quantize to bf16, stage + colsum, AllGather.

                lhs_tiles: list of SBUF tiles [128, RPC] f32 holding d^T (kparts k-chunks)
                rhs_w:     SBUF tile [128, kparts * f_out] f32
                """
                cs_ps = ps.tile([1, f_out], f32, name=f"cs{idx}", tag="z", bufs=6)
                r0 = 0
                for m, mt in enumerate(MTS):
                    yp = ps.tile([128, f_out], f32, name=f"yp{idx}_{m}", tag="y", bufs=2)
                    for kk in range(kparts):
                        nc.tensor.matmul(
                            yp[0:mt, :],
                            lhs_tiles[kk][:, r0:r0 + mt],
                            rhs_w[:, kk * f_out:(kk + 1) * f_out],
                            start=(kk == 0), stop=(kk == kparts - 1))
                    yst = stage.tile([128, f_out], bf16, name=f"yst{idx}_{m}",
                                     tag="yst", bufs=3)
                    nc.scalar.copy(yst[0:mt, :], yp[0:mt, :])
                    dq = stage.tile([128, f_out], bf16, name=f"dq{idx}_{m}",
                                    tag="dq", bufs=3)
                    nc.vector.tensor_sub(dq[0:mt, :], yp[0:mt, :], yst[0:mt, :])
                    nc.tensor.matmul(cs_ps[:], ones_col[0:mt, :], dq[0:mt, :],
                                     start=(m == 0), stop=(m == len(MTS) - 1))
                    nc.sync.dma_start(yg_in[idx][r0:r0 + mt, :], yst[0:mt, :])
                    r0 += mt
                # zero padding rows + colsum row
                nc.sync.dma_start(yg_in[idx][RPC:KBLK, :], zrow[0:KBLK - RPC, 0:f_out])
                csr = stage.tile([1, f_out], bf16, name=f"csr{idx}", tag="csr", bufs=2)
                nc.scalar.copy(csr[:], cs_ps[:])
                nc.sync.dma_start(yg_in[idx][KBLK:BLK, :], csr[:])
                nc.gpsimd.collective_compute("AllGather", ALU.bypass, replica_groups=rg,
                                             ins=[yg_in[idx][:]], outs=[yg_out[idx][:]])

            # ---------- prologue: y0 ----------
            produce_y(0, HID, xk, W0s, 4)

            # ---------- masks (f32) ----------
            m0a = mp.tile([128, RPC], f32, name="m0a", tag="m0a")
            m0b = mp.tile([128, RPC], f32, name="m0b", tag="m0b")
            m1a = mp.tile([128, RPC], f32, name="m1a", tag="m1a")
            nc.sync.dma_start(m0a[:], m0T[0:128, :])
            nc.sync.dma_start(m0b[:], m0T[128:256, :])
            nc.sync.dma_start(m1a[:], m1T[:])

            d1a = dp.tile([128, RPC], f32, name="d1a", tag="d1a")
            d1b = dp.tile([128, RPC], f32, name="d1b", tag="d1b")
            d2a = dp.tile([128, RPC], f32, name="d2a", tag="d2a")

            # ---------- layers ----------
            layer_cfg = [
                # (f_in, feat tiles (partitions), bias tile slices, y index)
                (HID, [128, 128], 0),
                (MID, [128], 1),
                (OUT_C, [64], 2),
            ]

            z2_sb = const.tile([64, RPC], f32, name="z2sb")

            for li, (f_in, ftiles, yidx) in enumerate(layer_cfg):
                # global colsum of quantization residual (K=8 over per-core colsum rows)
                csums = stage.tile([8, f_in], bf16, name=f"csums{li}", tag="csums", bufs=2)
                nc.sync.dma_start(
                    csums[:],
                    yg_out[yidx].rearrange("(c r) f -> c r f", r=BLK)[:, KBLK, :])
                csg_ps = ps.tile([1, f_in], f32, name=f"csg{li}", tag="z", bufs=6)
                nc.tensor.matmul(csg_ps[:], ones_col[0:8, :], csums[:],
                                 start=True, stop=True)
                csg = stage.tile([1, f_in], bf16, name=f"csg{li}", tag="csg", bufs=2)
                nc.scalar.copy(csg[:], csg_ps[:])

                # z^T accumulation over 80 k-tiles
                zp = []
                for mf, pf in enumerate(ftiles):
                    for i, (o, n) in enumerate(NT):
                        zp.append(ps.tile([128, n], f32, name=f"z{li}_{mf}_{i}",
                                          tag="z", bufs=6))

                for k in range(KT):
                    if k < R_RES:
                        ar = res_tiles[k]
                    else:
                        ar = stream.tile([128, RPC], bf16, name=f"as{li}_{k}",
                                         tag="adjstream", bufs=STREAM_BUFS)
                        nc.sync.dma_start(ar[:], adjT[k * 128:(k + 1) * 128, :])
                    c, t = divmod(k, 10)
                    yk_t = stream.tile([128, f_in], bf16, name=f"yk{li}_{k}",
                                       tag="yk", bufs=YK_BUFS)
                    nc.sync.dma_start(
                        yk_t[:], yg_out[yidx][c * BLK + t * 128: c * BLK + (t + 1) * 128, :])
                    for mf, pf in enumerate(ftiles):
                        for i, (o, n) in enumerate(NT):
                            nc.tensor.matmul(
                                zp[mf * 3 + i][0:pf, :],
                                yk_t[:, mf * 128:mf * 128 + pf],
                                ar[:, o:o + n],
                                start=(k == 0), stop=False)
                # rank-1 compensation: rowsum_est_i * colmean_f, closes each accum group
                for mf, pf in enumerate(ftiles):
                    for i, (o, n) in enumerate(NT):
                        nc.tensor.matmul(
                            zp[mf * 3 + i][0:pf, :],
                            csg[:, mf * 128:mf * 128 + pf],
                            rowsumT[:, o:o + n],
                            start=False, stop=True)

                # epilogue
                if li == 0:
                    for mf, (dst, msk) in enumerate([(d1a, m0a), (d1b, m0b)]):
                        for i, (o, n) in enumerate(NT):
                            nc.scalar.activation(dst[:, o:o + n], zp[mf * 3 + i][:],
                                                 AF.Identity, bias=b0s[:, mf:mf + 1])
                        nc.vector.tensor_mul(dst[:], dst[:], msk[:])
                    produce_y(1, MID, [d1a, d1b], W1s, 2)
                elif li == 1:
                    for i, (o, n) in enumerate(NT):
                        nc.scalar.activation(d2a[:, o:o + n], zp[i][:],
                                             AF.Identity, bias=b1s[:])
                    nc.vector.tensor_mul(d2a[:], d2a[:], m1a[:])
                    produce_y(2, OUT_C, [d2a], W2s, 1)
                else:
                    for i, (o, n) in enumerate(NT):
                        nc.scalar.activation(z2_sb[:, o:o + n], zp[i][0:64, :],
                                             AF.Identity, bias=b2s[:])
                    # final log_softmax over features: transpose row tiles, reduce
                    r0 = 0
                    for m, mt in enumerate(MTS):
                        tp = ps.tile([128, 64], f32, name=f"tp{m}", tag="y", bufs=2)
                        nc.tensor.transpose(tp[0:mt, :], z2_sb[:, r0:r0 + mt],
                                            ident[:])
                        mx = stage.tile([128, 1], f32, name=f"mx{m}", tag="mx", bufs=3)
                        nc.vector.reduce_max(mx[0:mt, :], tp[0:mt, :], axis=AX.X,
                                             negate=True)
                        esc = stage.tile([128, 64], f32, name=f"esc{m}", tag="esc", bufs=3)
                        sacc = stage.tile([128, 1], f32, name=f"sacc{m}", tag="sacc", bufs=3)
                        nc.scalar.activation(esc[0:mt, :], tp[0:mt, :], AF.Exp,
                                             bias=mx[0:mt, :], accum_out=sacc[0:mt, :])
                        lss = stage.tile([128, 1], f32, name=f"lss{m}", tag="lss", bufs=3)
                        nc.scalar.activation(lss[0:mt, :], sacc[0:mt, :], AF.Ln)
                        nls = stage.tile([128, 1], f32, name=f"nls{m}", tag="nls", bufs=3)
                        nc.vector.tensor_sub(nls[0:mt, :], mx[0:mt, :], lss[0:mt, :])
                        osb = stage.tile([128, 64], f32, name=f"osb{m}", tag="osb", bufs=3)
                        nc.scalar.activation(osb[0:mt, :], tp[0:mt, :], AF.Identity,
                                             bias=nls[0:mt, :])
                        nc.sync.dma_start(outd[r0:r0 + mt, :], osb[0:mt, :])
                        r0 += mt

    nc.compile()
    return nc


def _get_nc():
    global _BUILT
    if _BUILT is None:
        _BUILT = _build()
    return _BUILT


def _make_masks():
    import jax
    dkey = jax.random.key(42)
    k0, k1 = jax.random.split(dkey)
    m0 = np.asarray(jax.random.bernoulli(k0, 0.5, (N, HID))).astype(np.float32) * 2.0
    m1 = np.asarray(jax.random.bernoulli(k1, 0.5, (N, MID))).astype(np.float32) * 2.0
    return m0, m1


def prepare_in_maps(x, adj, W0, b0, W1, b1, W2, b2):
    x = np.asarray(x, dtype=np.float32)
    adj = np.asarray(adj, dtype=np.float32)
    W0 = np.asarray(W0, dtype=np.float32)
    W1 = np.asarray(W1, dtype=np.float32)
    W2 = np.asarray(W2, dtype=np.float32)
    b0 = np.asarray(b0, dtype=np.float32).reshape(HID, 1)
    b1 = np.asarray(b1, dtype=np.float32).reshape(MID, 1)
    b2 = np.asarray(b2, dtype=np.float32).reshape(OUT_C, 1)

    m0, m1 = _make_masks()
    adj_bf = adj.astype(ml_dtypes.bfloat16)

    in_maps = []
    for c in range(NCORES):
        rc = slice(c * RPC, (c + 1) * RPC)
        adjT_pad = np.zeros((KPAD, RPC), dtype=ml_dtypes.bfloat16)
        for b in range(NCORES):
            adjT_pad[b * KBLK:b * KBLK + RPC, :] = \
                adj_bf[rc, b * RPC:(b + 1) * RPC].T
        in_maps.append({
            "adjT": adjT_pad,
            "xT": np.ascontiguousarray(x[rc, :].T),
            "m0T": np.ascontiguousarray(m0[rc, :].T),
            "m1T": np.ascontiguousarray(m1[rc, :].T),
            "W0": W0, "W1": W1, "W2": W2,
            "b0": b0, "b1": b1, "b2": b2,
        })
    return in_maps


def kernel(x, adj, W0, b0, W1, b1, W2, b2, _run_kwargs=None):
    nc = _get_nc()
    in_maps = prepare_in_maps(x, adj, W0, b0, W1, b1, W2, b2)
    res = run_bass_kernel_spmd(nc, in_maps, list(range(NCORES)), **(_run_kwargs or {}))
    out = np.concatenate([res.results[c]["out"] for c in range(NCORES)], axis=0)
    kernel.last_results = res
    return out


if __name__ == "__main__":
    rng = np.random.default_rng(0)
    ins = {
        "x": rng.standard_normal((N, IN_C)).astype(np.float32),
        "adj": rng.random((N, N)).astype(np.float32),
        "W0": (rng.standard_normal((IN_C, HID)) / np.sqrt(IN_C)).astype(np.float32),
        "b0": np.zeros(HID, np.float32),
        "W1": (rng.standard_normal((HID, MID)) / np.sqrt(HID)).astype(np.float32),
        "b1": np.zeros(MID, np.float32),
        "W2": (rng.standard_normal((MID, OUT_C)) / np.sqrt(MID)).astype(np.float32),
        "b2": np.zeros(OUT_C, np.float32),
    }
    out = kernel(**ins)
    print("out", out.shape, out.dtype, out[:2, :4])


# revision 10
# speedup vs baseline: 1.0495x; 1.0495x over previous
"""Trainium2 Bass kernel for nn_BiGCN (3-layer GCN: batchnorm -> 3x [adj @ (x W) + b] with
dropout between layers, final log_softmax).

Strategy (8 NeuronCores, SPMD):
  - Row-shard adj over N: core c owns rows [c*1250, (c+1)*1250).
  - Host pre-transposes: each core receives adjT_c = adj[rows_c, :].T  (shape [N, 1250]),
    zero-padded per 1250-column block to 1280 so the contraction dim tiles evenly by 128,
    cast to bf16.  x is sent transposed ([512, 1250] slice, fp32).
  - On device everything is computed in the "transposed" orientation z^T = y^T @ adjT so
    per-feature bias/normalization are per-partition ops, and the y = d @ W matmuls take d^T
    directly as lhsT (no transposes anywhere except the final log_softmax).
  - Between layers, each core computes its row-block of y = d @ W in fp32, quantizes to bf16
    for the big adj matmul, and the blocks are exchanged with an on-device AllGather.
  - Numerical structure: adj has near-constant row sums (~N/2), which amplify the column-mean
    of any quantization error of y by ~100x per layer (compounding).  We therefore keep the
    d/y path in fp32 and add a rank-1 compensation rowsum_i * colmean(y - yq)_f to each adj
    matmul (one K=1 matmul accumulated into the same PSUM bank).  Validated to ~2e-3 final
    relative error vs the fp32 reference (naive bf16 is ~0.2).
  - Batchnorm statistics use a tiny AllReduce of per-core partial sums.
"""

import numpy as np
import ml_dtypes

import concourse.bass as bass
import concourse.bacc as bacc
import concourse.mybir as mybir
from concourse import tile, masks
from concourse.bass_utils import run_bass_kernel_spmd

f32 = mybir.dt.float32
bf16 = mybir.dt.bfloat16
AF = mybir.ActivationFunctionType
ALU = mybir.AluOpType
AX = mybir.AxisListType

# ---- problem shape (hardcoded per spec) ----
N = 10000
IN_C, HID, MID, OUT_C = 512, 256, 128, 64
EPS = 1e-5
NCORES = 8
RPC = N // NCORES          # 1250 rows per core
KBLK = 1280                # padded contraction rows per core block (10 k-tiles)
KPAD = NCORES * KBLK       # 10240
KT = KPAD // 128           # 80 k-tiles
BLK = KBLK + 1             # gather block: 1280 y rows (1250 real + 30 zero) + 1 colsum row
NT = [(0, 512), (512, 512), (1024, 226)]      # free-dim tiles of the 1250 row block
MTS = [128] * 9 + [98]                        # row tiles of the 1250 rows (sum = 1250)

# ---- tunables ----
R_RES = 28                 # adj k-tiles kept resident in SBUF across all three layers (even)
PAIR = 2                   # adj k-tiles per streamed DMA
STREAM_BUFS = 4
YBLK_BUFS = 4


def _covered_real(r_res: int) -> int:
    cov = 0
    for k in range(r_res):
        t = k % 10
        lo, hi = t * 128, min(t * 128 + 128, RPC)
        cov += max(0, hi - lo)
    return cov


COVERED = _covered_real(R_RES)

_BUILT = None


def _build():
    nc = bacc.Bacc("TRN2", target_bir_lowering=False, debug=False, num_devices=NCORES)

    adjT = nc.dram_tensor("adjT", [KPAD, RPC], bf16, kind="ExternalInput")
    xT = nc.dram_tensor("xT", [IN_C, RPC], f32, kind="ExternalInput")
    m0T = nc.dram_tensor("m0T", [HID, RPC], f32, kind="ExternalInput")
    m1T = nc.dram_tensor("m1T", [MID, RPC], f32, kind="ExternalInput")
    W0d = nc.dram_tensor("W0", [IN_C, HID], f32, kind="ExternalInput")
    W1d = nc.dram_tensor("W1", [HID, MID], f32, kind="ExternalInput")
    W2d = nc.dram_tensor("W2", [MID, OUT_C], f32, kind="ExternalInput")
    b0d = nc.dram_tensor("b0", [HID, 1], f32, kind="ExternalInput")
    b1d = nc.dram_tensor("b1", [MID, 1], f32, kind="ExternalInput")
    b2d = nc.dram_tensor("b2", [OUT_C, 1], f32, kind="ExternalInput")
    outd = nc.dram_tensor("out", [RPC, OUT_C], f32, kind="ExternalOutput")

    with tile.TileContext(nc) as tc:
        with (
            tc.tile_pool(name="const", bufs=1) as const,
            tc.tile_pool(name="xp", bufs=1) as xp,
            tc.tile_pool(name="dp", bufs=1) as dp,
            tc.tile_pool(name="mp", bufs=1) as mp,
            tc.tile_pool(name="res", bufs=1) as res,
            tc.tile_pool(name="stream", bufs=1) as stream,
            tc.tile_pool(name="stage", bufs=1) as stage,
            tc.tile_pool(name="ps", space="PSUM", bufs=1) as ps,
            tc.tile_pool(name="dram", space="DRAM", bufs=1) as dram,
        ):
            rg = [list(range(NCORES))]

            # ---------- batchnorm statistics (front of the critical path) ----------
            xk = []
            for i in range(4):
                t = xp.tile([128, RPC], f32, name=f"xk{i}", tag=f"xk{i}")
                nc.sync.dma_start(t[:], xT[i * 128:(i + 1) * 128, :])
                xk.append(t)

            stats = const.tile([128, 8], f32)
            sq = dp.tile([128, RPC], f32, name="sq", tag="d1a")
            for i in range(4):
                nc.vector.reduce_sum(stats[:, i:i + 1], xk[i][:], axis=AX.X)
                nc.scalar.activation(sq[:], xk[i][:], AF.Square,
                                     accum_out=stats[:, 4 + i:5 + i])

            bn_in = dram.tile([128, 8], f32)
            bn_out = dram.tile([128, 8], f32, addr_space="Shared")
            nc.sync.dma_start(bn_in[:], stats[:])
            nc.gpsimd.collective_compute("AllReduce", ALU.add, replica_groups=rg,
                                         ins=[bn_in[:]], outs=[bn_out[:]])
            statsr = const.tile([128, 8], f32)
            nc.sync.dma_start(statsr[:], bn_out[:])

            meanv = const.tile([128, 4], f32)
            ex2 = const.tile([128, 4], f32)
            var = const.tile([128, 4], f32)
            sd = const.tile([128, 4], f32)
            inv = const.tile([128, 4], f32)
            shift = const.tile([128, 4], f32)
            nc.vector.tensor_scalar_mul(meanv[:], statsr[:, 0:4], 1.0 / N)
            nc.vector.tensor_scalar_mul(ex2[:], statsr[:, 4:8], 1.0 / N)
            nc.vector.tensor_mul(var[:], meanv[:], meanv[:])
            nc.vector.tensor_sub(var[:], ex2[:], var[:])
            nc.vector.tensor_scalar_add(var[:], var[:], EPS)
            nc.scalar.activation(sd[:], var[:], AF.Sqrt)
            nc.vector.reciprocal(inv[:], sd[:])
            nc.vector.tensor_mul(shift[:], meanv[:], inv[:])
            nc.vector.tensor_scalar_mul(shift[:], shift[:], -1.0)

            # normalize x in place (per-partition scale/bias)
            for i in range(4):
                nc.scalar.activation(xk[i][:], xk[i][:], AF.Identity,
                                     scale=inv[:, i:i + 1], bias=shift[:, i:i + 1])

            # ---------- constants ----------
            ident = const.tile([64, 64], f32)
            masks.make_identity(nc, ident[:])
            ones_col = const.tile([128, 1], bf16)
            nc.gpsimd.memset(ones_col[:], 1.0)
            zrow = const.tile([32, HID], bf16)
            nc.gpsimd.memset(zrow[:], 0.0)

            W0s = const.tile([128, 4 * HID], f32)
            for kk in range(4):
                nc.sync.dma_start(W0s[:, kk * HID:(kk + 1) * HID],
                                  W0d[kk * 128:(kk + 1) * 128, :])
            W1s = const.tile([128, 2 * MID], f32)
            for kk in range(2):
                nc.sync.dma_start(W1s[:, kk * MID:(kk + 1) * MID],
                                  W1d[kk * 128:(kk + 1) * 128, :])
            W2s = const.tile([128, OUT_C], f32)
            nc.sync.dma_start(W2s[:], W2d[:])
            b0s = const.tile([128, 2], f32)
            nc.sync.dma_start(b0s[:, 0:1], b0d[0:128, :])
            nc.sync.dma_start(b0s[:, 1:2], b0d[128:256, :])
            b1s = const.tile([128, 1], f32)
            nc.sync.dma_start(b1s[:], b1d[:])
            b2s = const.tile([64, 1], f32)
            nc.sync.dma_start(b2s[:], b2d[:])

            # ---------- resident adj block (one big tile) + partial rowsum ----------
            ares = res.tile([128, R_RES * RPC], bf16, name="ares")
            FILL = 4  # k-tiles per resident-fill DMA
            for k0 in range(0, R_RES, FILL):
                nc.gpsimd.dma_start(
                    ares[:, k0 * RPC:(k0 + FILL) * RPC]
                        .rearrange("p (t f) -> p t f", t=FILL),
                    adjT[k0 * 128:(k0 + FILL) * 128, :]
                        .rearrange("(t p) f -> p t f", p=128))

            rs_ps = [ps.tile([1, n], f32, name=f"rsps{i}", tag="z", bufs=6)
                     for i, (o, n) in enumerate(NT)]
            for k in range(R_RES):
                ark = ares[:, k * RPC:(k + 1) * RPC]
                for i, (o, n) in enumerate(NT):
                    nc.tensor.matmul(rs_ps[i][:], ones_col[:, :], ark[:, o:o + n],
                                     start=(k == 0), stop=(k == R_RES - 1))
            # compensation = rowsum_est_i * colmean_f = (partial_i*N/COVERED) * (colsum_f/N)
            rowsumT = const.tile([1, RPC], bf16)
            for i, (o, n) in enumerate(NT):
                nc.scalar.activation(rowsumT[:, o:o + n], rs_ps[i][:], AF.Copy,
                                     scale=1.0 / COVERED)

            # ---------- gather buffers ----------
            yg_in = [dram.tile([BLK, HID], bf16, name="ygi0"),
                     dram.tile([BLK, MID], bf16, name="ygi1"),
                     dram.tile([BLK, OUT_C], bf16, name="ygi2")]
            yg_out = [dram.tile([NCORES * BLK, HID], bf16, addr_space="Shared", name="ygo0"),
                      dram.tile([NCORES * BLK, MID], bf16, addr_space="Shared", name="ygo1"),
                      dram.tile([NCORES * BLK, OUT_C], bf16, addr_space="Shared", name="ygo2")]

            def produce_y(idx, f_out, lhs_tiles, rhs_w, kparts):
                """Compute y = d @ W (fp32), quantize to bf16, stage + colsum, AllGather."""
                cs_ps = ps.tile([1, f_out], f32, name=f"cs{idx}", tag="z", bufs=6)
                r0 = 0
                for m, mt in enumerate(MTS):
                    yp = ps.tile([128, f_out], f32, name=f"yp{idx}_{m}", tag="y", bufs=2)
                    for kk in range(kparts):
                        nc.tensor.matmul(
                            yp[0:mt, :],
                            lhs_tiles[kk][:, r0:r0 + mt],
                            rhs_w[:, kk * f_out:(kk + 1) * f_out],
                            start=(kk == 0), stop=(kk == kparts - 1))
                    yst = stage.tile([128, f_out], bf16, name=f"yst{idx}_{m}",
                                     tag="yst", bufs=3)
                    nc.scalar.copy(yst[0:mt, :], yp[0:mt, :])
                    dq = stage.tile([128, f_out], bf16, name=f"dq{idx}_{m}",
                                    tag="dq", bufs=3)
                    nc.vector.tensor_sub(dq[0:mt, :], yp[0:mt, :], yst[0:mt, :])
                    nc.tensor.matmul(cs_ps[:], ones_col[0:mt, :], dq[0:mt, :],
                                     start=(m == 0), stop=(m == len(MTS) - 1))
                    nc.scalar.dma_start(yg_in[idx][r0:r0 + mt, :], yst[0:mt, :])
                    r0 += mt
                # zero padding rows + colsum row
                nc.scalar.dma_start(yg_in[idx][RPC:KBLK, :], zrow[0:KBLK - RPC, 0:f_out])
                csr = stage.tile([1, f_out], bf16, name=f"csr{idx}", tag="csr", bufs=2)
                nc.scalar.copy(csr[:], cs_ps[:])
                nc.scalar.dma_start(yg_in[idx][KBLK:BLK, :], csr[:])
                nc.gpsimd.collective_compute("AllGather", ALU.bypass, replica_groups=rg,
                                             ins=[yg_in[idx][:]], outs=[yg_out[idx][:]])

            # ---------- prologue: y0 ----------
            produce_y(0, HID, xk, W0s, 4)

            # ---------- masks (f32) ----------
            m0a = mp.tile([128, RPC], f32, name="m0a", tag="m0a")
            m0b = mp.tile([128, RPC], f32, name="m0b", tag="m0b")
            m1a = mp.tile([128, RPC], f32, name="m1a", tag="m1a")
            nc.scalar.dma_start(m0a[:], m0T[0:128, :])
            nc.scalar.dma_start(m0b[:], m0T[128:256, :])
            nc.scalar.dma_start(m1a[:], m1T[:])

            d1a = dp.tile([128, RPC], f32, name="d1a", tag="d1a")
            d1b = dp.tile([128, RPC], f32, name="d1b", tag="d1b")
            d2a = dp.tile([128, RPC], f32, name="d2a", tag="d2a")

            layer_cfg = [
                (HID, [128, 128], 0),
                (MID, [128], 1),
                (OUT_C, [64], 2),
            ]

            z2_sb = const.tile([64, RPC], f32, name="z2sb")

            for li, (f_in, ftiles, yidx) in enumerate(layer_cfg):
                # global colsum of quantization residual (K=8 over per-core colsum rows)
                csums = stage.tile([8, f_in], bf16, name=f"csums{li}", tag="csums", bufs=2)
                nc.sync.dma_start(
                    csums[:],
                    yg_out[yidx].rearrange("(c r) f -> c r f", r=BLK)[:, KBLK, :])
                csg_ps = ps.tile([1, f_in], f32, name=f"csg{li}", tag="z", bufs=6)
                nc.tensor.matmul(csg_ps[:], ones_col[0:8, :], csums[:],
                                 start=True, stop=True)
                csg = stage.tile([1, f_in], bf16, name=f"csg{li}", tag="csg", bufs=2)
                nc.scalar.copy(csg[:], csg_ps[:])

                # z^T accumulation over 80 k-tiles
                zp = []
                for mf, pf in enumerate(ftiles):
                    for i, (o, n) in enumerate(NT):
                        zp.append(ps.tile([128, n], f32, name=f"z{li}_{mf}_{i}",
                                          tag="z", bufs=6))

                yblks = []
                for c in range(NCORES):
                    yb = stream.tile([128, 10 * f_in], bf16, name=f"yb{li}_{c}",
                                     tag="yblk", bufs=YBLK_BUFS)
                    nc.sync.dma_start(
                        yb[:].rearrange("p (t f) -> p t f", t=10),
                        yg_out[yidx][c * BLK:c * BLK + KBLK, :]
                            .rearrange("(t p) f -> p t f", p=128))
                    yblks.append(yb)

                def z_mms(k, ar_ap):
                    c, t = divmod(k, 10)
                    for mf, pf in enumerate(ftiles):
                        lhs = yblks[c][:, t * f_in + mf * 128: t * f_in + mf * 128 + pf]
                        for i, (o, n) in enumerate(NT):
                            nc.tensor.matmul(
                                zp[mf * 3 + i][0:pf, :], lhs, ar_ap[:, o:o + n],
                                start=(k == 0), stop=False)

                for k in range(R_RES):
                    z_mms(k, ares[:, k * RPC:(k + 1) * RPC])
                for k0 in range(R_RES, KT, PAIR):
                    arp = stream.tile([128, PAIR * RPC], bf16, name=f"as{li}_{k0}",
                                      tag="adjstream", bufs=STREAM_BUFS)
                    nc.gpsimd.dma_start(
                        arp[:].rearrange("p (t f) -> p t f", t=PAIR),
                        adjT[k0 * 128:(k0 + PAIR) * 128, :]
                            .rearrange("(t p) f -> p t f", p=128))
                    for j in range(PAIR):
                        z_mms(k0 + j, arp[:, j * RPC:(j + 1) * RPC])
                # rank-1 compensation closes each accumulation group
                for mf, pf in enumerate(ftiles):
                    for i, (o, n) in enumerate(NT):
                        nc.tensor.matmul(
                            zp[mf * 3 + i][0:pf, :],
                            csg[:, mf * 128:mf * 128 + pf],
                            rowsumT[:, o:o + n],
                            start=False, stop=True)

                # epilogue
                if li == 0:
                    for mf, (dst, msk) in enumerate([(d1a, m0a), (d1b, m0b)]):
                        for i, (o, n) in enumerate(NT):
                            nc.scalar.activation(dst[:, o:o + n], zp[mf * 3 + i][:],
                                                 AF.Identity, bias=b0s[:, mf:mf + 1])
                        nc.vector.tensor_mul(dst[:], dst[:], msk[:])
                    produce_y(1, MID, [d1a, d1b], W1s, 2)
                elif li == 1:
                    for i, (o, n) in enumerate(NT):
                        nc.scalar.activation(d2a[:, o:o + n], zp[i][:],
                                             AF.Identity, bias=b1s[:])
                    nc.vector.tensor_mul(d2a[:], d2a[:], m1a[:])
                    produce_y(2, OUT_C, [d2a], W2s, 1)
                else:
                    for i, (o, n) in enumerate(NT):
                        nc.scalar.activation(z2_sb[:, o:o + n], zp[i][0:64, :],
                                             AF.Identity, bias=b2s[:])
                    # final log_softmax over features: transpose row tiles, reduce
                    r0 = 0
                    for m, mt in enumerate(MTS):
                        tp = ps.tile([128, 64], f32, name=f"tp{m}", tag="y", bufs=2)
                        nc.tensor.transpose(tp[0:mt, :], z2_sb[:, r0:r0 + mt],
                                            ident[:])
                        mx = stage.tile([128, 1], f32, name=f"mx{m}", tag="mx", bufs=3)
                        nc.vector.reduce_max(mx[0:mt, :], tp[0:mt, :], axis=AX.X,
                                             negate=True)
                        esc = stage.tile([128, 64], f32, name=f"esc{m}", tag="esc", bufs=3)
                        sacc = stage.tile([128, 1], f32, name=f"sacc{m}", tag="sacc", bufs=3)
                        nc.scalar.activation(esc[0:mt, :], tp[0:mt, :], AF.Exp,
                                             bias=mx[0:mt, :], accum_out=sacc[0:mt, :])
                        lss = stage.tile([128, 1], f32, name=f"lss{m}", tag="lss", bufs=3)
                        nc.scalar.activation(lss[0:mt, :], sacc[0:mt, :], AF.Ln)
                        nls = stage.tile([128, 1], f32, name=f"nls{m}", tag="nls", bufs=3)
                        nc.vector.tensor_sub(nls[0:mt, :], mx[0:mt, :], lss[0:mt, :])
                        osb = stage.tile([128, 64], f32, name=f"osb{m}", tag="osb", bufs=3)
                        nc.scalar.activation(osb[0:mt, :], tp[0:mt, :], AF.Identity,
                                             bias=nls[0:mt, :])
                        nc.scalar.dma_start(outd[r0:r0 + mt, :], osb[0:mt, :])
                        r0 += mt

    nc.compile()
    return nc


def _get_nc():
    global _BUILT
    if _BUILT is None:
        _BUILT = _build()
    return _BUILT


def _make_masks():
    import jax
    dkey = jax.random.key(42)
    k0, k1 = jax.random.split(dkey)
    m0 = np.asarray(jax.random.bernoulli(k0, 0.5, (N, HID))).astype(np.float32) * 2.0
    m1 = np.asarray(jax.random.bernoulli(k1, 0.5, (N, MID))).astype(np.float32) * 2.0
    return m0, m1


def prepare_in_maps(x, adj, W0, b0, W1, b1, W2, b2):
    x = np.asarray(x, dtype=np.float32)
    adj = np.asarray(adj, dtype=np.float32)
    W0 = np.asarray(W0, dtype=np.float32)
    W1 = np.asarray(W1, dtype=np.float32)
    W2 = np.asarray(W2, dtype=np.float32)
    b0 = np.asarray(b0, dtype=np.float32).reshape(HID, 1)
    b1 = np.asarray(b1, dtype=np.float32).reshape(MID, 1)
    b2 = np.asarray(b2, dtype=np.float32).reshape(OUT_C, 1)

    m0, m1 = _make_masks()
    adj_bf = adj.astype(ml_dtypes.bfloat16)

    in_maps = []
    for c in range(NCORES):
        rc = slice(c * RPC, (c + 1) * RPC)
        adjT_pad = np.zeros((KPAD, RPC), dtype=ml_dtypes.bfloat16)
        for b in range(NCORES):
            adjT_pad[b * KBLK:b * KBLK + RPC, :] = \
                adj_bf[rc, b * RPC:(b + 1) * RPC].T
        in_maps.append({
            "adjT": adjT_pad,
            "xT": np.ascontiguousarray(x[rc, :].T),
            "m0T": np.ascontiguousarray(m0[rc, :].T),
            "m1T": np.ascontiguousarray(m1[rc, :].T),
            "W0": W0, "W1": W1, "W2": W2,
            "b0": b0, "b1": b1, "b2": b2,
        })
    return in_maps


def kernel(x, adj, W0, b0, W1, b1, W2, b2, _run_kwargs=None):
    nc = _get_nc()
    in_maps = prepare_in_maps(x, adj, W0, b0, W1, b1, W2, b2)
    res = run_bass_kernel_spmd(nc, in_maps, list(range(NCORES)), **(_run_kwargs or {}))
    out = np.concatenate([res.results[c]["out"] for c in range(NCORES)], axis=0)
    kernel.last_results = res
    return out


if __name__ == "__main__":
    rng = np.random.default_rng(0)
    ins = {
        "x": rng.standard_normal((N, IN_C)).astype(np.float32),
        "adj": rng.random((N, N)).astype(np.float32),
        "W0": (rng.standard_normal((IN_C, HID)) / np.sqrt(IN_C)).astype(np.float32),
        "b0": np.zeros(HID, np.float32),
        "W1": (rng.standard_normal((HID, MID)) / np.sqrt(HID)).astype(np.float32),
        "b1": np.zeros(MID, np.float32),
        "W2": (rng.standard_normal((MID, OUT_C)) / np.sqrt(MID)).astype(np.float32),
        "b2": np.zeros(OUT_C, np.float32),
    }
    out = kernel(**ins)
    print("out", out.shape, out.dtype, out[:2, :4])
